# revision 10
# baseline (speedup 1.0000x reference)
"""Dcls1d (dilated conv with learnable spacings, depthwise) Trainium2 kernel.

Problem: x [16, 256, 8192] f32, depthwise conv per channel with a 56-wide
kernel holding 7 interpolated taps (positions = k*8+4 + P, linear interp),
padding 27/27, plus bias.  Output [16, 256, 8191] f32.

Strategy:
  - Data-parallel over batch: 2 images per NeuronCore (8 cores).
  - Host precomputes, per channel c and tap k: integer base shift i0[c,k]
    and the two interpolation coefficients a[c,k] (weight*(1-r)) and
    b[c,k] (weight*r) reading x_pad at offsets i0 and i0+1.
  - Per core the padded input lives in DRAM as 512 rows (2 batches x 256
    channels) of width 8256: [27 zeros][8192 data][37 zeros] so every
    shifted read is in-bounds and the zeros realize the conv padding.
  - For each (channel-group, chunk) tile the kernel issues 7 indirect
    DMA gathers (per-partition row offsets = row*8256 + i0[c,k]) into
    SBUF, then runs 14 scalar_tensor_tensor multiply-accumulate passes
    on the Vector engine (per-partition scalar coefficients), and one
    tensor_scalar pass folding the bias.
"""

import os
from contextlib import ExitStack

import numpy as np

import concourse.bass as bass
import concourse.bacc as bacc_mod
import concourse.mybir as mybir
import concourse.tile as tile
from concourse.bass_utils import run_bass_kernel_spmd

# Problem geometry (hardcoded per spec nn_Dcls1d_12713103196284)
N, C, L = 16, 256, 8192
OUT_L = 8191
KS, DIL, PAD = 7, 8, 27
LK = DIL * KS  # 56
N_CORES = 8
NB = N // N_CORES  # batches per core
ROWS = NB * C  # 512 rows per core
PADW = 8256  # [27 zeros][8192][37 zeros], 64B-aligned rows
CHUNK = 2048
CHUNKS = [(0, 2048), (2048, 2048), (4096, 2048), (6144, 2047)]
GROUPS_PER_C = C // 128  # 2
NTILES = NB * GROUPS_PER_C  # 4

F32 = mybir.dt.float32
F16 = mybir.dt.float16
I32 = mybir.dt.int32
SUB = 512  # matmul subchunk (one PSUM bank of f32)

_PROG = None
_PROG_IMPL = None
LAST_RESULTS = None  # test harness reads exec_time_ns off this


def _build_program_pe():
    """TensorE variant: fp16 gathers; per (tap, a/b) a diagonal 128x128 fp16
    lhsT scales the shifted slice per-channel and accumulates into PSUM
    (fp32); ScalarE evacuates PSUM with the bias add; one DMA store per
    2048-chunk."""
    nc = bacc_mod.Bacc()
    xpad = nc.dram_tensor("xpad", [ROWS, PADW], F16, kind="ExternalInput")
    idx = nc.dram_tensor("idx", [128, NTILES * KS], I32, kind="ExternalInput")
    diags = nc.dram_tensor(
        "diags", [128, GROUPS_PER_C * KS * 2 * 128], F16, kind="ExternalInput"
    )
    cbias = nc.dram_tensor("cbias", [128, GROUPS_PER_C], F32, kind="ExternalInput")
    out = nc.dram_tensor("out", [ROWS, OUT_L], F32, kind="ExternalOutput")

    with ExitStack() as ctx:
        tc = ctx.enter_context(tile.TileContext(nc))
        const = ctx.enter_context(tc.tile_pool(name="const", bufs=1))
        idx_sb = const.tile([128, NTILES * KS], I32)
        nc.sync.dma_start(idx_sb[:], idx[:])
        diag_sb = const.tile([128, GROUPS_PER_C * KS * 2 * 128], F16)
        nc.sync.dma_start(diag_sb[:], diags[:])
        cbias_sb = const.tile([128, GROUPS_PER_C], F32)
        nc.sync.dma_start(cbias_sb[:], cbias[:])

        xs_pool = ctx.enter_context(tc.tile_pool(name="xs", bufs=2))
        psum_pool = ctx.enter_context(
            tc.tile_pool(name="ps", bufs=8, space="PSUM")
        )
        ev_pool = ctx.enter_context(tc.tile_pool(name="ev", bufs=2))

        for t in range(NTILES):
            b, g = divmod(t, GROUPS_PER_C)
            row0 = b * C + g * 128
            for c0, w in CHUNKS:
                xs = [
                    xs_pool.tile([128, CHUNK + 1], F16, tag=f"xs{k}", name=f"xs{k}")
                    for k in range(KS)
                ]
                for k in range(KS):
                    col = t * KS + k
                    nc.gpsimd.indirect_dma_start(
                        out=xs[k][:, 0 : w + 1],
                        out_offset=None,
                        in_=xpad[:],
                        in_offset=bass.IndirectOffsetOnAxis(
                            ap=idx_sb[:, col : col + 1], axis=1
                        ),
                        element_offset=c0,
                    )
                ev = ev_pool.tile([128, CHUNK], F32)
                for s in range(CHUNK // SUB):
                    s0 = s * SUB
                    sw = min(SUB, w - s0)
                    ps = psum_pool.tile([128, SUB], F32)
                    for k in range(KS):
                        j = (g * KS + k) * 2
                        nc.tensor.matmul(
                            out=ps[:, 0:sw],
                            lhsT=diag_sb[:, j * 128 : (j + 1) * 128],
                            rhs=xs[k][:, s0 : s0 + sw],
                            start=(k == 0),
                            stop=False,
                        )
                        nc.tensor.matmul(
                            out=ps[:, 0:sw],
                            lhsT=diag_sb[:, (j + 1) * 128 : (j + 2) * 128],
                            rhs=xs[k][:, s0 + 1 : s0 + 1 + sw],
                            start=False,
                            stop=(k == KS - 1),
                        )
                    nc.scalar.activation(
                        ev[:, s0 : s0 + sw],
                        ps[:, 0:sw],
                        mybir.ActivationFunctionType.Identity,
                        bias=cbias_sb[:, g : g + 1],
                        scale=1.0,
                    )
                nc.sync.dma_start(out[row0 : row0 + 128, c0 : c0 + w], ev[:, 0:w])
    nc.finalize()
    return nc


CHUNK2 = 4096
CHUNKS2 = [(0, 4096), (4096, 4095)]
DVE_MOD = 3  # every DVE_MOD-th 512-subchunk runs on VectorE instead of PE


def _build_program_pe2():
    """Like _build_program_pe, but: fp16 output stores, 4096-wide chunks,
    and every third 512-subchunk computed on the (otherwise idle) Vector
    engine via fp16 scalar_tensor_tensor chains to relieve both the DMA
    (smaller stores) and TensorE (fewer matmuls)."""
    nc = bacc_mod.Bacc()
    xpad = nc.dram_tensor("xpad", [ROWS, PADW], F16, kind="ExternalInput")
    idx = nc.dram_tensor("idx", [128, NTILES * KS], I32, kind="ExternalInput")
    diags = nc.dram_tensor(
        "diags", [128, GROUPS_PER_C * KS * 2 * 128], F16, kind="ExternalInput"
    )
    ca = nc.dram_tensor("ca", [128, GROUPS_PER_C * KS], F32, kind="ExternalInput")
    cb = nc.dram_tensor("cb", [128, GROUPS_PER_C * KS], F32, kind="ExternalInput")
    cbias = nc.dram_tensor("cbias", [128, GROUPS_PER_C], F32, kind="ExternalInput")
    out = nc.dram_tensor("out", [ROWS, OUT_L], F16, kind="ExternalOutput")

    mult = mybir.AluOpType.mult
    add = mybir.AluOpType.add

    with ExitStack() as ctx:
        tc = ctx.enter_context(tile.TileContext(nc))
        const = ctx.enter_context(tc.tile_pool(name="const", bufs=1))
        idx_sb = const.tile([128, NTILES * KS], I32)
        nc.sync.dma_start(idx_sb[:], idx[:])
        diag_sb = const.tile([128, GROUPS_PER_C * KS * 2 * 128], F16)
        nc.sync.dma_start(diag_sb[:], diags[:])
        ca_sb = const.tile([128, GROUPS_PER_C * KS], F32)
        nc.sync.dma_start(ca_sb[:], ca[:])
        cb_sb = const.tile([128, GROUPS_PER_C * KS], F32)
        nc.sync.dma_start(cb_sb[:], cb[:])
        cbias_sb = const.tile([128, GROUPS_PER_C], F32)
        nc.sync.dma_start(cbias_sb[:], cbias[:])

        xs_pool = ctx.enter_context(tc.tile_pool(name="xs", bufs=2))
        psum_pool = ctx.enter_context(tc.tile_pool(name="ps", bufs=8, space="PSUM"))
        ev_pool = ctx.enter_context(tc.tile_pool(name="ev", bufs=2))

        sub_i = 0
        for t in range(NTILES):
            b, g = divmod(t, GROUPS_PER_C)
            row0 = b * C + g * 128
            for c0, w in CHUNKS2:
                xs = [
                    xs_pool.tile(
                        [128, CHUNK2 + 1], F16, tag=f"xs{k}", name=f"xs{k}"
                    )
                    for k in range(KS)
                ]
                for k in range(KS):
                    col = t * KS + k
                    nc.gpsimd.indirect_dma_start(
                        out=xs[k][:, 0 : w + 1],
                        out_offset=None,
                        in_=xpad[:],
                        in_offset=bass.IndirectOffsetOnAxis(
                            ap=idx_sb[:, col : col + 1], axis=1
                        ),
                        element_offset=c0,
                    )
                ev = ev_pool.tile([128, CHUNK2], F16)
                cc = g * KS
                for s in range(CHUNK2 // SUB):
                    s0 = s * SUB
                    sw = min(SUB, w - s0)
                    evs = ev[:, s0 : s0 + sw]
                    if sub_i % DVE_MOD == DVE_MOD - 1:
                        # VectorE subchunk: fp16 multiply-accumulate chain
                        nc.vector.tensor_scalar(
                            evs,
                            xs[0][:, s0 : s0 + sw],
                            ca_sb[:, cc : cc + 1],
                            cbias_sb[:, g : g + 1],
                            mult,
                            add,
                        )
                        nc.vector.scalar_tensor_tensor(
                            evs,
                            xs[0][:, s0 + 1 : s0 + 1 + sw],
                            cb_sb[:, cc : cc + 1],
                            evs,
                            mult,
                            add,
                        )
                        for k in range(1, KS):
                            nc.vector.scalar_tensor_tensor(
                                evs,
                                xs[k][:, s0 : s0 + sw],
                                ca_sb[:, cc + k : cc + k + 1],
                                evs,
                                mult,
                                add,
                            )
                            nc.vector.scalar_tensor_tensor(
                                evs,
                                xs[k][:, s0 + 1 : s0 + 1 + sw],
                                cb_sb[:, cc + k : cc + k + 1],
                                evs,
                                mult,
                                add,
                            )
                    else:
                        ps = psum_pool.tile([128, SUB], F32)
                        for k in range(KS):
                            j = (g * KS + k) * 2
                            nc.tensor.matmul(
                                out=ps[:, 0:sw],
                                lhsT=diag_sb[:, j * 128 : (j + 1) * 128],
                                rhs=xs[k][:, s0 : s0 + sw],
                                start=(k == 0),
                                stop=False,
                            )
                            nc.tensor.matmul(
                                out=ps[:, 0:sw],
                                lhsT=diag_sb[:, (j + 1) * 128 : (j + 2) * 128],
                                rhs=xs[k][:, s0 + 1 : s0 + 1 + sw],
                                start=False,
                                stop=(k == KS - 1),
                            )
                        nc.scalar.activation(
                            evs,
                            ps[:, 0:sw],
                            mybir.ActivationFunctionType.Identity,
                            bias=cbias_sb[:, g : g + 1],
                            scale=1.0,
                        )
                    sub_i += 1
                nc.sync.dma_start(out[row0 : row0 + 128, c0 : c0 + w], ev[:, 0:w])
    nc.finalize()
    return nc


def _build_program():
    nc = bacc_mod.Bacc()
    xpad = nc.dram_tensor("xpad", [ROWS, PADW], F32, kind="ExternalInput")
    idx = nc.dram_tensor("idx", [128, NTILES * KS], I32, kind="ExternalInput")
    ca = nc.dram_tensor("ca", [128, GROUPS_PER_C * KS], F32, kind="ExternalInput")
    cb = nc.dram_tensor("cb", [128, GROUPS_PER_C * KS], F32, kind="ExternalInput")
    cbias = nc.dram_tensor("cbias", [128, GROUPS_PER_C], F32, kind="ExternalInput")
    out = nc.dram_tensor("out", [ROWS, OUT_L], F32, kind="ExternalOutput")

    mult = mybir.AluOpType.mult
    add = mybir.AluOpType.add

    with ExitStack() as ctx:
        tc = ctx.enter_context(tile.TileContext(nc))
        const = ctx.enter_context(tc.tile_pool(name="const", bufs=1))
        idx_sb = const.tile([128, NTILES * KS], I32)
        nc.sync.dma_start(idx_sb[:], idx[:])
        ca_sb = const.tile([128, GROUPS_PER_C * KS], F32)
        nc.sync.dma_start(ca_sb[:], ca[:])
        cb_sb = const.tile([128, GROUPS_PER_C * KS], F32)
        nc.sync.dma_start(cb_sb[:], cb[:])
        cbias_sb = const.tile([128, GROUPS_PER_C], F32)
        nc.sync.dma_start(cbias_sb[:], cbias[:])

        xs_pool = ctx.enter_context(tc.tile_pool(name="xs", bufs=2))
        acc_pool = ctx.enter_context(tc.tile_pool(name="acc", bufs=3))

        for t in range(NTILES):
            b, g = divmod(t, GROUPS_PER_C)
            row0 = b * C + g * 128
            for c0, w in CHUNKS:
                xs = [
                    xs_pool.tile([128, CHUNK + 1], F32, tag=f"xs{k}", name=f"xs{k}")
                    for k in range(KS)
                ]
                for k in range(KS):
                    col = t * KS + k
                    nc.gpsimd.indirect_dma_start(
                        out=xs[k][:, 0 : w + 1],
                        out_offset=None,
                        in_=xpad[:],
                        in_offset=bass.IndirectOffsetOnAxis(
                            ap=idx_sb[:, col : col + 1], axis=1
                        ),
                        element_offset=c0,
                    )
                acc = acc_pool.tile([128, CHUNK], F32)
                cc = g * KS
                # acc = xs0 * a0 + bias
                nc.vector.tensor_scalar(
                    acc[:, 0:w],
                    xs[0][:, 0:w],
                    ca_sb[:, cc : cc + 1],
                    cbias_sb[:, g : g + 1],
                    mult,
                    add,
                )
                nc.vector.scalar_tensor_tensor(
                    acc[:, 0:w],
                    xs[0][:, 1 : w + 1],
                    cb_sb[:, cc : cc + 1],
                    acc[:, 0:w],
                    mult,
                    add,
                )
                for k in range(1, KS):
                    nc.vector.scalar_tensor_tensor(
                        acc[:, 0:w],
                        xs[k][:, 0:w],
                        ca_sb[:, cc + k : cc + k + 1],
                        acc[:, 0:w],
                        mult,
                        add,
                    )
                    nc.vector.scalar_tensor_tensor(
                        acc[:, 0:w],
                        xs[k][:, 1 : w + 1],
                        cb_sb[:, cc + k : cc + k + 1],
                        acc[:, 0:w],
                        mult,
                        add,
                    )
                nc.sync.dma_start(out[row0 : row0 + 128, c0 : c0 + w], acc[:, 0:w])
    nc.finalize()
    return nc


def _host_taps(weight, P):
    """Mirror reference.construct_kernel's float32 math: per (channel, tap)
    integer shift i0 into the 27-padded row and coefficients a (at i0) and
    b (at i0+1)."""
    w = np.asarray(weight, dtype=np.float32)[:, 0, :]  # [C, KS]
    Pm = np.asarray(P, dtype=np.float32)[0, :, 0, :]  # [C, KS]
    base = (np.arange(KS, dtype=np.float32) * DIL + DIL // 2).astype(np.float32)
    p = np.clip(Pm + base[None, :], np.float32(0.0), np.float32(LK - 1))
    i0f = np.floor(p)
    r = (p - i0f).astype(np.float32)
    i0 = i0f.astype(np.int32)
    i1 = np.minimum(i0 + 1, LK - 1)
    a = (w * (np.float32(1.0) - r)).astype(np.float32)
    bcoef = (w * r).astype(np.float32)
    clipped = i1 == i0  # i0 == 55: both interp points coincide
    a = np.where(clipped, a + bcoef, a)
    bcoef = np.where(clipped, np.float32(0.0), bcoef)
    return i0, a, bcoef


def kernel(x, weight, P, bias):
    global _PROG, _PROG_IMPL, LAST_RESULTS
    impl = os.environ.get("KERNEL_IMPL", "pe")
    x = np.ascontiguousarray(np.asarray(x, dtype=np.float32))
    bias = np.asarray(bias, dtype=np.float32)
    i0, a, b = _host_taps(weight, P)

    # Per-partition constant tables (identical on every core).
    idx_arr = np.zeros((128, NTILES * KS), dtype=np.int32)
    ca_arr = np.zeros((128, GROUPS_PER_C * KS), dtype=np.float32)
    cb_arr = np.zeros((128, GROUPS_PER_C * KS), dtype=np.float32)
    cbias_arr = np.zeros((128, GROUPS_PER_C), dtype=np.float32)
    for t in range(NTILES):
        bt, g = divmod(t, GROUPS_PER_C)
        row0 = bt * C + g * 128
        ch = g * 128 + np.arange(128)
        for k in range(KS):
            idx_arr[:, t * KS + k] = (row0 + np.arange(128)) * PADW + i0[ch, k]
    for g in range(GROUPS_PER_C):
        ch = g * 128 + np.arange(128)
        for k in range(KS):
            ca_arr[:, g * KS + k] = a[ch, k]
            cb_arr[:, g * KS + k] = b[ch, k]
        cbias_arr[:, g] = bias[ch]

    # Pad per-core shards: rows [27 zeros][8192][37 zeros].
    xr = x.reshape(N_CORES, ROWS, L)
    xdt = np.float16 if impl in ("pe", "pe2") else np.float32
    xpad_all = np.zeros((N_CORES, ROWS, PADW), dtype=xdt)
    xpad_all[:, :, PAD : PAD + L] = xr

    if _PROG is None or _PROG_IMPL != impl:
        builders = {"pe": _build_program_pe, "pe2": _build_program_pe2, "dve": _build_program}
        _PROG = builders[impl]()
        _PROG_IMPL = impl
    nc = _PROG

    if impl in ("pe", "pe2"):
        diag_arr = np.zeros((128, GROUPS_PER_C * KS * 2 * 128), dtype=np.float16)
        rows128 = np.arange(128)
        for g in range(GROUPS_PER_C):
            ch = g * 128 + rows128
            for k in range(KS):
                j = (g * KS + k) * 2
                diag_arr[rows128, j * 128 + rows128] = a[ch, k].astype(np.float16)
                diag_arr[rows128, (j + 1) * 128 + rows128] = b[ch, k].astype(
                    np.float16
                )
        in_maps = [
            {
                "xpad": xpad_all[i],
                "idx": idx_arr,
                "diags": diag_arr,
                "cbias": cbias_arr,
            }
            for i in range(N_CORES)
        ]
        if impl == "pe2":
            for m in in_maps:
                m["ca"] = ca_arr
                m["cb"] = cb_arr
    else:
        in_maps = [
            {
                "xpad": xpad_all[i],
                "idx": idx_arr,
                "ca": ca_arr,
                "cb": cb_arr,
                "cbias": cbias_arr,
            }
            for i in range(N_CORES)
        ]
    trace = bool(int(os.environ.get("KERNEL_TRACE", "0")))
    res = run_bass_kernel_spmd(nc, in_maps, list(range(N_CORES)), trace=trace)
    LAST_RESULTS = res
    out = np.concatenate(
        [res.results[i]["out"].reshape(NB, C, OUT_L) for i in range(N_CORES)], axis=0
    )
    return np.ascontiguousarray(out.astype(np.float32))


# revision 11
# speedup vs baseline: 1.1841x; 1.1841x over previous
"""Dcls1d (dilated conv with learnable spacings, depthwise) Trainium2 kernel.

Problem: x [16, 256, 8192] f32, depthwise conv per channel with a 56-wide
kernel holding 7 interpolated taps (positions = k*8+4 + P, linear interp),
padding 27/27, plus bias.  Output [16, 256, 8191] f32.

Strategy:
  - Data-parallel over batch: 2 images per NeuronCore (8 cores).
  - Host precomputes, per channel c and tap k: integer base shift i0[c,k]
    and the two interpolation coefficients a[c,k] (weight*(1-r)) and
    b[c,k] (weight*r) reading x_pad at offsets i0 and i0+1.
  - Per core the padded input lives in DRAM as 512 rows (2 batches x 256
    channels) of width 8256: [27 zeros][8192 data][37 zeros] so every
    shifted read is in-bounds and the zeros realize the conv padding.
  - For each (channel-group, chunk) tile the kernel issues 7 indirect
    DMA gathers (per-partition row offsets = row*8256 + i0[c,k]) into
    SBUF, then runs 14 scalar_tensor_tensor multiply-accumulate passes
    on the Vector engine (per-partition scalar coefficients), and one
    tensor_scalar pass folding the bias.
"""

import os
from contextlib import ExitStack

import numpy as np

import concourse.bass as bass
import concourse.bacc as bacc_mod
import concourse.mybir as mybir
import concourse.tile as tile
from concourse.bass_utils import run_bass_kernel_spmd

# Problem geometry (hardcoded per spec nn_Dcls1d_12713103196284)
N, C, L = 16, 256, 8192
OUT_L = 8191
KS, DIL, PAD = 7, 8, 27
LK = DIL * KS  # 56
N_CORES = 8
NB = N // N_CORES  # batches per core
ROWS = NB * C  # 512 rows per core
PADW = 8256  # [27 zeros][8192][37 zeros], 64B-aligned rows
CHUNK = 2048
CHUNKS = [(0, 2048), (2048, 2048), (4096, 2048), (6144, 2047)]
GROUPS_PER_C = C // 128  # 2
NTILES = NB * GROUPS_PER_C  # 4

F32 = mybir.dt.float32
F16 = mybir.dt.float16
I32 = mybir.dt.int32
SUB = 512  # matmul subchunk (one PSUM bank of f32)

_PROG = None
_PROG_IMPL = None
LAST_RESULTS = None  # test harness reads exec_time_ns off this


def _build_program_pe():
    """TensorE variant: fp16 gathers; per (tap, a/b) a diagonal 128x128 fp16
    lhsT scales the shifted slice per-channel and accumulates into PSUM
    (fp32); ScalarE evacuates PSUM with the bias add; one DMA store per
    2048-chunk."""
    nc = bacc_mod.Bacc()
    xpad = nc.dram_tensor("xpad", [ROWS, PADW], F16, kind="ExternalInput")
    idx = nc.dram_tensor("idx", [128, NTILES * KS], I32, kind="ExternalInput")
    diags = nc.dram_tensor(
        "diags", [128, GROUPS_PER_C * KS * 2 * 128], F16, kind="ExternalInput"
    )
    cbias = nc.dram_tensor("cbias", [128, GROUPS_PER_C], F32, kind="ExternalInput")
    out = nc.dram_tensor("out", [ROWS, OUT_L], F32, kind="ExternalOutput")

    with ExitStack() as ctx:
        tc = ctx.enter_context(tile.TileContext(nc))
        const = ctx.enter_context(tc.tile_pool(name="const", bufs=1))
        idx_sb = const.tile([128, NTILES * KS], I32)
        nc.sync.dma_start(idx_sb[:], idx[:])
        diag_sb = const.tile([128, GROUPS_PER_C * KS * 2 * 128], F16)
        nc.sync.dma_start(diag_sb[:], diags[:])
        cbias_sb = const.tile([128, GROUPS_PER_C], F32)
        nc.sync.dma_start(cbias_sb[:], cbias[:])

        xs_pool = ctx.enter_context(tc.tile_pool(name="xs", bufs=2))
        psum_pool = ctx.enter_context(
            tc.tile_pool(name="ps", bufs=8, space="PSUM")
        )
        ev_pool = ctx.enter_context(tc.tile_pool(name="ev", bufs=2))

        for t in range(NTILES):
            b, g = divmod(t, GROUPS_PER_C)
            row0 = b * C + g * 128
            for c0, w in CHUNKS:
                xs = [
                    xs_pool.tile([128, CHUNK + 1], F16, tag=f"xs{k}", name=f"xs{k}")
                    for k in range(KS)
                ]
                for k in range(KS):
                    col = t * KS + k
                    nc.gpsimd.indirect_dma_start(
                        out=xs[k][:, 0 : w + 1],
                        out_offset=None,
                        in_=xpad[:],
                        in_offset=bass.IndirectOffsetOnAxis(
                            ap=idx_sb[:, col : col + 1], axis=1
                        ),
                        element_offset=c0,
                    )
                ev = ev_pool.tile([128, CHUNK], F32)
                for s in range(CHUNK // SUB):
                    s0 = s * SUB
                    sw = min(SUB, w - s0)
                    ps = psum_pool.tile([128, SUB], F32)
                    for k in range(KS):
                        j = (g * KS + k) * 2
                        nc.tensor.matmul(
                            out=ps[:, 0:sw],
                            lhsT=diag_sb[:, j * 128 : (j + 1) * 128],
                            rhs=xs[k][:, s0 : s0 + sw],
                            start=(k == 0),
                            stop=False,
                        )
                        nc.tensor.matmul(
                            out=ps[:, 0:sw],
                            lhsT=diag_sb[:, (j + 1) * 128 : (j + 2) * 128],
                            rhs=xs[k][:, s0 + 1 : s0 + 1 + sw],
                            start=False,
                            stop=(k == KS - 1),
                        )
                    nc.scalar.activation(
                        ev[:, s0 : s0 + sw],
                        ps[:, 0:sw],
                        mybir.ActivationFunctionType.Identity,
                        bias=cbias_sb[:, g : g + 1],
                        scale=1.0,
                    )
                nc.sync.dma_start(out[row0 : row0 + 128, c0 : c0 + w], ev[:, 0:w])
    nc.finalize()
    return nc


CHUNK2 = 4096
CHUNKS2 = [(0, 4096), (4096, 4095)]
DVE_MOD = 5  # every DVE_MOD-th 512-subchunk runs on VectorE instead of PE


def _build_program_pe2():
    """Like _build_program_pe, but: fp16 output stores, 4096-wide chunks,
    and every third 512-subchunk computed on the (otherwise idle) Vector
    engine via fp16 scalar_tensor_tensor chains to relieve both the DMA
    (smaller stores) and TensorE (fewer matmuls)."""
    nc = bacc_mod.Bacc()
    xpad = nc.dram_tensor("xpad", [ROWS, PADW], F16, kind="ExternalInput")
    idx = nc.dram_tensor("idx", [128, NTILES * KS], I32, kind="ExternalInput")
    diags = nc.dram_tensor(
        "diags", [128, GROUPS_PER_C * KS * 2 * 128], F16, kind="ExternalInput"
    )
    ca = nc.dram_tensor("ca", [128, GROUPS_PER_C * KS], F32, kind="ExternalInput")
    cb = nc.dram_tensor("cb", [128, GROUPS_PER_C * KS], F32, kind="ExternalInput")
    cbias = nc.dram_tensor("cbias", [128, GROUPS_PER_C], F32, kind="ExternalInput")
    out = nc.dram_tensor("out", [ROWS, OUT_L], F16, kind="ExternalOutput")

    mult = mybir.AluOpType.mult
    add = mybir.AluOpType.add

    with ExitStack() as ctx:
        tc = ctx.enter_context(tile.TileContext(nc))
        const = ctx.enter_context(tc.tile_pool(name="const", bufs=1))
        idx_sb = const.tile([128, NTILES * KS], I32)
        nc.sync.dma_start(idx_sb[:], idx[:])
        diag_sb = const.tile([128, GROUPS_PER_C * KS * 2 * 128], F16)
        nc.sync.dma_start(diag_sb[:], diags[:])
        ca_sb = const.tile([128, GROUPS_PER_C * KS], F32)
        nc.sync.dma_start(ca_sb[:], ca[:])
        cb_sb = const.tile([128, GROUPS_PER_C * KS], F32)
        nc.sync.dma_start(cb_sb[:], cb[:])
        cbias_sb = const.tile([128, GROUPS_PER_C], F32)
        nc.sync.dma_start(cbias_sb[:], cbias[:])

        xs_pool = ctx.enter_context(tc.tile_pool(name="xs", bufs=2))
        psum_pool = ctx.enter_context(tc.tile_pool(name="ps", bufs=8, space="PSUM"))
        ev_pool = ctx.enter_context(tc.tile_pool(name="ev", bufs=2))

        sub_i = 0
        for t in range(NTILES):
            b, g = divmod(t, GROUPS_PER_C)
            row0 = b * C + g * 128
            for c0, w in CHUNKS2:
                xs = [
                    xs_pool.tile(
                        [128, CHUNK2 + 1], F16, tag=f"xs{k}", name=f"xs{k}"
                    )
                    for k in range(KS)
                ]
                for k in range(KS):
                    col = t * KS + k
                    nc.gpsimd.indirect_dma_start(
                        out=xs[k][:, 0 : w + 1],
                        out_offset=None,
                        in_=xpad[:],
                        in_offset=bass.IndirectOffsetOnAxis(
                            ap=idx_sb[:, col : col + 1], axis=1
                        ),
                        element_offset=c0,
                    )
                ev = ev_pool.tile([128, CHUNK2], F16)
                cc = g * KS
                for s in range(CHUNK2 // SUB):
                    s0 = s * SUB
                    sw = min(SUB, w - s0)
                    evs = ev[:, s0 : s0 + sw]
                    if sub_i % DVE_MOD == DVE_MOD - 1:
                        # VectorE subchunk: multiply-accumulate chain into a
                        # fp32 PSUM accumulator (STT runs 1x regardless, so
                        # the fp32 accumulator costs nothing extra and keeps
                        # PE-path precision); ScalarE evacuates.
                        pd = psum_pool.tile([128, SUB], F32, name="pd", tag="ps")
                        pda = pd[:, 0:sw]
                        nc.vector.tensor_scalar(
                            pda,
                            xs[0][:, s0 : s0 + sw],
                            ca_sb[:, cc : cc + 1],
                            cbias_sb[:, g : g + 1],
                            mult,
                            add,
                        )
                        nc.vector.scalar_tensor_tensor(
                            pda,
                            xs[0][:, s0 + 1 : s0 + 1 + sw],
                            cb_sb[:, cc : cc + 1],
                            pda,
                            mult,
                            add,
                        )
                        for k in range(1, KS):
                            nc.vector.scalar_tensor_tensor(
                                pda,
                                xs[k][:, s0 : s0 + sw],
                                ca_sb[:, cc + k : cc + k + 1],
                                pda,
                                mult,
                                add,
                            )
                            nc.vector.scalar_tensor_tensor(
                                pda,
                                xs[k][:, s0 + 1 : s0 + 1 + sw],
                                cb_sb[:, cc + k : cc + k + 1],
                                pda,
                                mult,
                                add,
                            )
                        nc.scalar.activation(
                            evs,
                            pda,
                            mybir.ActivationFunctionType.Copy,
                        )
                    else:
                        ps = psum_pool.tile([128, SUB], F32)
                        for k in range(KS):
                            j = (g * KS + k) * 2
                            nc.tensor.matmul(
                                out=ps[:, 0:sw],
                                lhsT=diag_sb[:, j * 128 : (j + 1) * 128],
                                rhs=xs[k][:, s0 : s0 + sw],
                                start=(k == 0),
                                stop=False,
                            )
                            nc.tensor.matmul(
                                out=ps[:, 0:sw],
                                lhsT=diag_sb[:, (j + 1) * 128 : (j + 2) * 128],
                                rhs=xs[k][:, s0 + 1 : s0 + 1 + sw],
                                start=False,
                                stop=(k == KS - 1),
                            )
                        nc.scalar.activation(
                            evs,
                            ps[:, 0:sw],
                            mybir.ActivationFunctionType.Identity,
                            bias=cbias_sb[:, g : g + 1],
                            scale=1.0,
                        )
                    sub_i += 1
                nc.sync.dma_start(out[row0 : row0 + 128, c0 : c0 + w], ev[:, 0:w])
    nc.finalize()
    return nc


def _build_program():
    nc = bacc_mod.Bacc()
    xpad = nc.dram_tensor("xpad", [ROWS, PADW], F32, kind="ExternalInput")
    idx = nc.dram_tensor("idx", [128, NTILES * KS], I32, kind="ExternalInput")
    ca = nc.dram_tensor("ca", [128, GROUPS_PER_C * KS], F32, kind="ExternalInput")
    cb = nc.dram_tensor("cb", [128, GROUPS_PER_C * KS], F32, kind="ExternalInput")
    cbias = nc.dram_tensor("cbias", [128, GROUPS_PER_C], F32, kind="ExternalInput")
    out = nc.dram_tensor("out", [ROWS, OUT_L], F32, kind="ExternalOutput")

    mult = mybir.AluOpType.mult
    add = mybir.AluOpType.add

    with ExitStack() as ctx:
        tc = ctx.enter_context(tile.TileContext(nc))
        const = ctx.enter_context(tc.tile_pool(name="const", bufs=1))
        idx_sb = const.tile([128, NTILES * KS], I32)
        nc.sync.dma_start(idx_sb[:], idx[:])
        ca_sb = const.tile([128, GROUPS_PER_C * KS], F32)
        nc.sync.dma_start(ca_sb[:], ca[:])
        cb_sb = const.tile([128, GROUPS_PER_C * KS], F32)
        nc.sync.dma_start(cb_sb[:], cb[:])
        cbias_sb = const.tile([128, GROUPS_PER_C], F32)
        nc.sync.dma_start(cbias_sb[:], cbias[:])

        xs_pool = ctx.enter_context(tc.tile_pool(name="xs", bufs=2))
        acc_pool = ctx.enter_context(tc.tile_pool(name="acc", bufs=3))

        for t in range(NTILES):
            b, g = divmod(t, GROUPS_PER_C)
            row0 = b * C + g * 128
            for c0, w in CHUNKS:
                xs = [
                    xs_pool.tile([128, CHUNK + 1], F32, tag=f"xs{k}", name=f"xs{k}")
                    for k in range(KS)
                ]
                for k in range(KS):
                    col = t * KS + k
                    nc.gpsimd.indirect_dma_start(
                        out=xs[k][:, 0 : w + 1],
                        out_offset=None,
                        in_=xpad[:],
                        in_offset=bass.IndirectOffsetOnAxis(
                            ap=idx_sb[:, col : col + 1], axis=1
                        ),
                        element_offset=c0,
                    )
                acc = acc_pool.tile([128, CHUNK], F32)
                cc = g * KS
                # acc = xs0 * a0 + bias
                nc.vector.tensor_scalar(
                    acc[:, 0:w],
                    xs[0][:, 0:w],
                    ca_sb[:, cc : cc + 1],
                    cbias_sb[:, g : g + 1],
                    mult,
                    add,
                )
                nc.vector.scalar_tensor_tensor(
                    acc[:, 0:w],
                    xs[0][:, 1 : w + 1],
                    cb_sb[:, cc : cc + 1],
                    acc[:, 0:w],
                    mult,
                    add,
                )
                for k in range(1, KS):
                    nc.vector.scalar_tensor_tensor(
                        acc[:, 0:w],
                        xs[k][:, 0:w],
                        ca_sb[:, cc + k : cc + k + 1],
                        acc[:, 0:w],
                        mult,
                        add,
                    )
                    nc.vector.scalar_tensor_tensor(
                        acc[:, 0:w],
                        xs[k][:, 1 : w + 1],
                        cb_sb[:, cc + k : cc + k + 1],
                        acc[:, 0:w],
                        mult,
                        add,
                    )
                nc.sync.dma_start(out[row0 : row0 + 128, c0 : c0 + w], acc[:, 0:w])
    nc.finalize()
    return nc


def _host_taps(weight, P):
    """Mirror reference.construct_kernel's float32 math: per (channel, tap)
    integer shift i0 into the 27-padded row and coefficients a (at i0) and
    b (at i0+1)."""
    w = np.asarray(weight, dtype=np.float32)[:, 0, :]  # [C, KS]
    Pm = np.asarray(P, dtype=np.float32)[0, :, 0, :]  # [C, KS]
    base = (np.arange(KS, dtype=np.float32) * DIL + DIL // 2).astype(np.float32)
    p = np.clip(Pm + base[None, :], np.float32(0.0), np.float32(LK - 1))
    i0f = np.floor(p)
    r = (p - i0f).astype(np.float32)
    i0 = i0f.astype(np.int32)
    i1 = np.minimum(i0 + 1, LK - 1)
    a = (w * (np.float32(1.0) - r)).astype(np.float32)
    bcoef = (w * r).astype(np.float32)
    clipped = i1 == i0  # i0 == 55: both interp points coincide
    a = np.where(clipped, a + bcoef, a)
    bcoef = np.where(clipped, np.float32(0.0), bcoef)
    return i0, a, bcoef


def kernel(x, weight, P, bias):
    global _PROG, _PROG_IMPL, LAST_RESULTS
    impl = os.environ.get("KERNEL_IMPL", "pe")
    x = np.ascontiguousarray(np.asarray(x, dtype=np.float32))
    bias = np.asarray(bias, dtype=np.float32)
    i0, a, b = _host_taps(weight, P)

    # Per-partition constant tables (identical on every core).
    idx_arr = np.zeros((128, NTILES * KS), dtype=np.int32)
    ca_arr = np.zeros((128, GROUPS_PER_C * KS), dtype=np.float32)
    cb_arr = np.zeros((128, GROUPS_PER_C * KS), dtype=np.float32)
    cbias_arr = np.zeros((128, GROUPS_PER_C), dtype=np.float32)
    for t in range(NTILES):
        bt, g = divmod(t, GROUPS_PER_C)
        row0 = bt * C + g * 128
        ch = g * 128 + np.arange(128)
        for k in range(KS):
            idx_arr[:, t * KS + k] = (row0 + np.arange(128)) * PADW + i0[ch, k]
    for g in range(GROUPS_PER_C):
        ch = g * 128 + np.arange(128)
        for k in range(KS):
            ca_arr[:, g * KS + k] = a[ch, k]
            cb_arr[:, g * KS + k] = b[ch, k]
        cbias_arr[:, g] = bias[ch]

    # Pad per-core shards: rows [27 zeros][8192][37 zeros].
    xr = x.reshape(N_CORES, ROWS, L)
    xdt = np.float16 if impl in ("pe", "pe2") else np.float32
    xpad_all = np.zeros((N_CORES, ROWS, PADW), dtype=xdt)
    xpad_all[:, :, PAD : PAD + L] = xr

    if _PROG is None or _PROG_IMPL != impl:
        builders = {"pe": _build_program_pe, "pe2": _build_program_pe2, "dve": _build_program}
        _PROG = builders[impl]()
        _PROG_IMPL = impl
    nc = _PROG

    if impl in ("pe", "pe2"):
        diag_arr = np.zeros((128, GROUPS_PER_C * KS * 2 * 128), dtype=np.float16)
        rows128 = np.arange(128)
        for g in range(GROUPS_PER_C):
            ch = g * 128 + rows128
            for k in range(KS):
                j = (g * KS + k) * 2
                diag_arr[rows128, j * 128 + rows128] = a[ch, k].astype(np.float16)
                diag_arr[rows128, (j + 1) * 128 + rows128] = b[ch, k].astype(
                    np.float16
                )
        in_maps = [
            {
                "xpad": xpad_all[i],
                "idx": idx_arr,
                "diags": diag_arr,
                "cbias": cbias_arr,
            }
            for i in range(N_CORES)
        ]
        if impl == "pe2":
            for m in in_maps:
                m["ca"] = ca_arr
                m["cb"] = cb_arr
    else:
        in_maps = [
            {
                "xpad": xpad_all[i],
                "idx": idx_arr,
                "ca": ca_arr,
                "cb": cb_arr,
                "cbias": cbias_arr,
            }
            for i in range(N_CORES)
        ]
    trace = bool(int(os.environ.get("KERNEL_TRACE", "0")))
    res = run_bass_kernel_spmd(nc, in_maps, list(range(N_CORES)), trace=trace)
    LAST_RESULTS = res
    out = np.concatenate(
        [res.results[i]["out"].reshape(NB, C, OUT_L) for i in range(N_CORES)], axis=0
    )
    return np.ascontiguousarray(out.astype(np.float32))
